# revision 12
# baseline (speedup 1.0000x reference)
"""AdaAttention Trainium2 kernel: 8-way batch data parallel.

Full inputs in, full outputs out. Each of the 8 NeuronCores processes a
128-row batch shard. Weights (~1.3M params) are replicated, host-packed to
bf16 tiles.

Per-core dataflow:
  att_feats [128,196,512] f32 --(gpsimd casting DMA)--> bf16 natural tiles
    --(XBAR dma transpose)--> [d,b] stationary chunks
  z[b,s,h] = att@W_ctx (4 MM) + h_emb via one identity-MM (PSUM accumulation)
  hA = tanh(z)                                (ScalarE, batched x2 slices)
  scores = sum_h hA*W_alpha                   (DVE mult + grouped reduce)
  online softmax: e = exp(scores - L); num accumulated ON PE via
    diag(e) stationary matmuls into a dedicated PSUM bank; den = sum e
  out = tanh((num/den + h_lin) @ W_att2h + b) (PE + DVE + ScalarE)
"""
import numpy as np
import ml_dtypes

B = 1024
NCORES = 8
BL = B // NCORES          # 128 rows per core
S = 196                   # attention positions
D = 512                   # feature dim (RNN=ENC=HID=512)
SC = 14                   # att slices per DMA chunk
NCHUNK = S // SC          # 14
GRP = 2                   # slices per PSUM group
NGRP = SC // GRP          # 7 groups per chunk
CHUNKS = [SC] * NCHUNK

BF16 = ml_dtypes.bfloat16

_CACHE = {}


def _pack_w(w):
    # [512,512] (in,out) -> [128, 4, 512]: tile[p, dc, o] = w[dc*128+p, o]
    return np.ascontiguousarray(
        w.reshape(4, 128, D).transpose(1, 0, 2)).astype(BF16)


def _build(has_bz, has_bz0, neg_l):
    import concourse.bass as bass
    import concourse.tile as tile
    from concourse import bacc, mybir
    from concourse.masks import make_identity

    f32 = mybir.dt.float32
    bf16 = mybir.dt.bfloat16
    AF = mybir.ActivationFunctionType

    nc = bacc.Bacc("TRN2", target_bir_lowering=False, debug=False,
                   num_devices=NCORES)

    h_ap = nc.dram_tensor("h", [BL, D], f32, kind="ExternalInput").ap()
    sent_ap = nc.dram_tensor("sent", [BL, D], f32, kind="ExternalInput").ap()
    att_ap = nc.dram_tensor("att", [BL, S, D], f32, kind="ExternalInput").ap()
    w_aps = {}
    for name in ("wsl", "whl", "wse", "whe", "wctx", "watt", "wa4"):
        w_aps[name] = nc.dram_tensor(name, [128, 4, D], bf16,
                                     kind="ExternalInput").ap()
    bsl_ap = nc.dram_tensor("bsl", [128, 4], f32, kind="ExternalInput").ap()
    bhl_ap = nc.dram_tensor("bhl", [128, 4], f32, kind="ExternalInput").ap()
    brep_ap = nc.dram_tensor("brep", [128, D], f32, kind="ExternalInput").ap()
    if has_bz:
        bz_ap = nc.dram_tensor("bz", [1, D], bf16, kind="ExternalInput").ap()
    if has_bz0:
        bz0_ap = nc.dram_tensor("bz0", [1, D], bf16, kind="ExternalInput").ap()
    out_ap = nc.dram_tensor("out", [BL, D], f32, kind="ExternalOutput").ap()

    with tile.TileContext(nc) as tc:
        with tc.tile_pool(name="const", bufs=1) as constp, \
             tc.tile_pool(name="attp", bufs=4) as attp, \
             tc.tile_pool(name="attT", bufs=3) as attTp, \
             tc.tile_pool(name="hAp", bufs=3) as hAp, \
             tc.tile_pool(name="wzp", bufs=3) as wzp, \
             tc.tile_pool(name="scp", bufs=3) as scp, \
             tc.tile_pool(name="diagp", bufs=4) as diagp, \
             tc.tile_pool(name="small", bufs=1) as smallp, \
             tc.tile_pool(name="psum", bufs=3, space="PSUM") as psump, \
             tc.tile_pool(name="psumt", bufs=1, space="PSUM") as psumtp, \
             tc.tile_pool(name="nacc", bufs=1, space="PSUM") as naccp:

            # ---- h/sent casts first (tiny), then prefetch att chunk 0 ----
            h_bf = smallp.tile([128, D], bf16, tag="hbf")
            nc.gpsimd.dma_start(out=h_bf[:], in_=h_ap[:])
            sent_bf = smallp.tile([128, D], bf16, tag="sentbf")
            nc.gpsimd.dma_start(out=sent_bf[:], in_=sent_ap[:])
            att_bf0 = attp.tile([128, SC, D], bf16, tag="attbf")
            nc.gpsimd.dma_start(out=att_bf0[:, 0:CHUNKS[0], :],
                                in_=att_ap[:, 0:CHUNKS[0], :])

            # ---- constants / weights ----
            w = {}
            for name in ("wsl", "whl", "wctx", "whe", "wse", "wa4", "watt"):
                t = constp.tile([128, 4, D], bf16, tag=name)
                nc.scalar.dma_start(out=t[:], in_=w_aps[name][:])
                w[name] = t
            bsl = constp.tile([128, 4], f32, tag="bsl")
            nc.scalar.dma_start(out=bsl[:], in_=bsl_ap[:])
            bhl = constp.tile([128, 4], f32, tag="bhl")
            nc.scalar.dma_start(out=bhl[:], in_=bhl_ap[:])
            brep = constp.tile([128, D], f32, tag="brep")
            nc.scalar.dma_start(out=brep[:], in_=brep_ap[:])
            bz = bz0 = None
            if has_bz:
                bz = constp.tile([1, D], bf16, tag="bz")
                nc.scalar.dma_start(out=bz[:], in_=bz_ap[:])
            if has_bz0:
                bz0 = constp.tile([1, D], bf16, tag="bz0")
                nc.scalar.dma_start(out=bz0[:], in_=bz0_ap[:])
            if has_bz or has_bz0:
                ones_row = constp.tile([1, 128], bf16, tag="ones")
                nc.vector.memset(ones_row[:], 1.0)
            ident = constp.tile([128, 128], bf16, tag="ident")
            make_identity(nc, ident[:])
            lshift = constp.tile([128, 1], f32, tag="lshift")
            nc.vector.memset(lshift[:], neg_l)
            e_all = constp.tile([128, 200], f32, tag="eall")

            # ---- h / sentinel branch ----
            hT = smallp.tile([128, 4, 128], bf16, tag="hT")
            nc.sync.dma_start_transpose(out=hT[:], in_=h_bf[:])
            sentT = smallp.tile([128, 4, 128], bf16, tag="sentT")
            nc.sync.dma_start_transpose(out=sentT[:], in_=sent_bf[:])

            def linear_T(xT, wt, bias_t, func, out_tag):
                """[e(P),b] = func(w[:,dc,:].T @ xT + b): transposed-layout
                linear layer. Returns bf16 [128, 4, 128] tile."""
                pt = psump.tile([128, GRP, D], f32, tag="z")
                ptf = pt.rearrange("p a b -> p (a b)")
                outT = smallp.tile([128, 4, 128], bf16, tag=out_tag)
                for ec in range(4):
                    reg = ptf[:, ec * 128:(ec + 1) * 128]
                    for dc in range(4):
                        nc.tensor.matmul(
                            reg, lhsT=wt[:, dc, ec * 128:(ec + 1) * 128],
                            rhs=xT[:, dc, :],
                            start=(dc == 0), stop=(dc == 3))
                    nc.scalar.activation(outT[:, ec, :], reg, func,
                                         bias=bias_t[:, ec:ec + 1])
                return outT

            sent_linT = linear_T(sentT, w["wsl"], bsl, AF.Relu, "slinT")
            h_linT = linear_T(hT, w["whl"], bhl, AF.Tanh, "hlinT")

            # natural-layout copies (b on partitions)
            ptn = psumtp.tile([128, GRP, D], bf16, tag="zt")
            ptn_flat = ptn.rearrange("p a b -> p (a b)")
            for ec in range(4):
                nc.tensor.transpose(ptn_flat[:, ec * 128:(ec + 1) * 128],
                                    sent_linT[:, ec, :], ident[:])
            for ec in range(4):
                nc.tensor.transpose(ptn_flat[:, D + ec * 128:D + (ec + 1) * 128],
                                    h_linT[:, ec, :], ident[:])
            sent_lin_nat = smallp.tile([128, D], bf16, tag="slnat")
            nc.scalar.copy(out=sent_lin_nat[:], in_=ptn_flat[:, 0:D])
            h_lin_nat = smallp.tile([128, D], f32, tag="hlnat")
            nc.scalar.copy(out=h_lin_nat[:], in_=ptn_flat[:, D:2 * D])

            # h_emb precomputed once in natural layout, injected per slice
            # with ONE identity matmul into the slice's PSUM.
            def make_hemb(extra_bias, tag):
                pt = psump.tile([128, GRP, D], f32, tag="z")
                reg = pt[:, 0, :]
                for ec in range(4):
                    nc.tensor.matmul(reg, lhsT=h_linT[:, ec, :],
                                     rhs=w["whe"][:, ec, :],
                                     start=(ec == 0),
                                     stop=(ec == 3 and extra_bias is None))
                if extra_bias is not None:
                    nc.tensor.matmul(reg, lhsT=ones_row[:],
                                     rhs=extra_bias[:], start=False, stop=True)
                out = smallp.tile([128, D], bf16, tag=tag)
                nc.scalar.copy(out=out[:], in_=reg)
                return out

            hemb_att = make_hemb(bz, "hembA")
            hemb_sent = make_hemb(bz0, "hembS")

            # num accumulator: one PSUM bank, PE-accumulated over 197 slices
            nacc = naccp.tile([128, D], f32, tag="nacc")

            def mac(e_col, src_nat, start, stop):
                dg = diagp.tile([128, 128], bf16, tag="diag")
                nc.gpsimd.affine_select(
                    out=dg[:], in_=e_col.to_broadcast((128, 128)),
                    compare_op=mybir.AluOpType.is_equal,
                    fill=0.0, base=0, pattern=[[-1, 128]],
                    channel_multiplier=1)
                nc.tensor.matmul(nacc[:], lhsT=dg[:], rhs=src_nat,
                                 start=start, stop=stop)

            # ---- sentinel slice (e-column 196) ----
            zt0 = psump.tile([128, GRP, D], f32, tag="z")
            reg0 = zt0[:, 0, :]
            for ec in range(4):
                nc.tensor.matmul(reg0, lhsT=sent_linT[:, ec, :],
                                 rhs=w["wse"][:, ec, :],
                                 start=(ec == 0), stop=False)
            nc.tensor.matmul(reg0, lhsT=ident[:], rhs=hemb_sent[:],
                             start=False, stop=True)
            hA0 = hAp.tile([128, GRP, D], bf16, tag="hA")
            nc.scalar.activation(hA0[:, 0, :], reg0, AF.Tanh)
            wz0 = wzp.tile([128, GRP, D], bf16, tag="wz")
            nc.vector.tensor_mul(wz0[:, 0, :], hA0[:, 0, :], w["wa4"][:, 0, :])
            sc0 = scp.tile([128, GRP], f32, tag="sc")
            nc.vector.reduce_sum(out=sc0[:, 0:1], in_=wz0[:, 0, :],
                                 axis=mybir.AxisListType.X)
            nc.scalar.activation(e_all[:, 196:197], sc0[:, 0:1], AF.Exp,
                                 bias=lshift[:])
            mac(e_all[:, 196:197], sent_lin_nat[:], start=True, stop=False)

            # ---- main loop over att slices (MAC pipelined 1 group behind) ----
            pending = []

            def flush_pending(last=False):
                limit = 0 if last else 3 * GRP
                while len(pending) > limit:
                    e_col, src = pending.pop(0)
                    mac(e_col, src, start=False,
                        stop=(last and not pending))

            NG = S // GRP                     # 98 groups total
            GPC = SC // GRP                   # groups per chunk
            PAIR = 2 * GRP                    # slices per DVE batch
            att_tiles = {}
            hA_pair = None
            for gi in range(NG):
                ck, g = divmod(gi, GPC)
                if g == 0:
                    if ck == 0:
                        att_bf = att_bf0
                    else:
                        att_bf = attp.tile([128, SC, D], bf16, tag="attbf")
                        nc.gpsimd.dma_start(
                            out=att_bf[:], in_=att_ap[:, ck * SC:(ck + 1) * SC, :])
                    att_tiles[ck] = att_bf
                    at_T = attTp.tile([128, SC * 4, 128], bf16, tag="attT")
                    nc.sync.dma_start_transpose(
                        out=at_T[:], in_=att_bf.rearrange("p a b -> p (a b)"))
                gl = g * GRP
                sg = gi * GRP
                zt = psump.tile([128, GRP, D], f32, tag="z")
                for j in range(GRP):
                    reg = zt[:, j, :]
                    for dc in range(4):
                        nc.tensor.matmul(
                            reg, lhsT=at_T[:, (gl + j) * 4 + dc, :],
                            rhs=w["wctx"][:, dc, :],
                            start=(dc == 0), stop=False)
                    nc.tensor.matmul(reg, lhsT=ident[:], rhs=hemb_att[:],
                                     start=False, stop=True)
                flush_pending()
                if gi % 2 == 0:
                    hA_pair = hAp.tile([128, PAIR, D], bf16, tag="hA")
                    nc.scalar.activation(hA_pair[:, 0:GRP, :], zt[:], AF.Tanh)
                else:
                    nc.scalar.activation(hA_pair[:, GRP:PAIR, :], zt[:], AF.Tanh)
                    wz = wzp.tile([128, PAIR, D], bf16, tag="wz")
                    nc.vector.tensor_mul(wz[:], hA_pair[:], w["wa4"][:, 0:PAIR, :])
                    sc4 = scp.tile([128, PAIR], f32, tag="sc")
                    nc.vector.reduce_sum(out=sc4[:], in_=wz[:],
                                         axis=mybir.AxisListType.X)
                    sp = sg - GRP
                    nc.scalar.activation(e_all[:, sp:sp + PAIR], sc4[:], AF.Exp,
                                         bias=lshift[:])
                    for j in range(PAIR):
                        sj = sp + j
                        pending.append((e_all[:, sj:sj + 1],
                                        att_tiles[sj // SC][:, sj % SC, :]))
            flush_pending(last=True)

            # ---- epilogue ----
            den = smallp.tile([128, 1], f32, tag="den")
            nc.vector.reduce_sum(out=den[:], in_=e_all[:, 0:197],
                                 axis=mybir.AxisListType.X)
            rec = smallp.tile([128, 1], f32, tag="rec")
            nc.vector.reciprocal(out=rec[:], in_=den[:])
            chat = smallp.tile([128, D], f32, tag="chat")
            nc.vector.tensor_scalar_mul(chat[:], nacc[:], rec[:])
            atten = smallp.tile([128, D], f32, tag="atten")
            nc.vector.tensor_add(atten[:], chat[:], h_lin_nat[:])
            atten_bf = smallp.tile([128, D], bf16, tag="attenbf")
            nc.vector.tensor_copy(out=atten_bf[:], in_=atten[:])
            ptf = psumtp.tile([128, GRP, D], bf16, tag="zt")
            ptf_flat = ptf.rearrange("p a b -> p (a b)")
            for dc in range(4):
                nc.tensor.transpose(ptf_flat[:, dc * 128:(dc + 1) * 128],
                                    atten_bf[:, dc * 128:(dc + 1) * 128],
                                    ident[:])
            attenT = smallp.tile([128, 4, 128], bf16, tag="attenT")
            nc.scalar.copy(out=attenT.rearrange("p a b -> p (a b)"),
                           in_=ptf_flat[:, 0:D])
            zf = psump.tile([128, GRP, D], f32, tag="z")
            regf = zf[:, 0, :]
            for dc in range(4):
                nc.tensor.matmul(regf, lhsT=attenT[:, dc, :],
                                 rhs=w["watt"][:, dc, :],
                                 start=(dc == 0), stop=(dc == 3))
            zfb = smallp.tile([128, D], f32, tag="zfb")
            nc.vector.tensor_add(zfb[:], regf, brep[:])
            out_sb = smallp.tile([128, D], f32, tag="outsb")
            nc.scalar.activation(out_sb[:], zfb[:], AF.Tanh)
            nc.scalar.dma_start(out=out_ap[:], in_=out_sb[:])

    nc.compile()
    return nc


def _prepare(h, sentinel, att_feats, W_ctx2att, b_ctx2att, W_sl, b_sl,
             W_se, b_se, W_hl, b_hl, W_he, b_he, W_alpha, b_alpha,
             W_att2h, b_att2h):
    h = np.asarray(h, dtype=np.float32)
    sentinel = np.asarray(sentinel, dtype=np.float32)
    att_feats = np.asarray(att_feats, dtype=np.float32)
    to_np = lambda a: np.asarray(a, dtype=np.float32)
    W_ctx2att, b_ctx2att = to_np(W_ctx2att), to_np(b_ctx2att)
    W_sl, b_sl = to_np(W_sl), to_np(b_sl)
    W_se, b_se = to_np(W_se), to_np(b_se)
    W_hl, b_hl = to_np(W_hl), to_np(b_hl)
    W_he, b_he = to_np(W_he), to_np(b_he)
    W_alpha, b_alpha = to_np(W_alpha), to_np(b_alpha)
    W_att2h, b_att2h = to_np(W_att2h), to_np(b_att2h)

    bias_z = b_ctx2att + b_he          # added to every att slice's z
    bias_z0 = b_se + b_he              # added to the sentinel slice's z
    has_bz = bool(np.any(bias_z))
    has_bz0 = bool(np.any(bias_z0))
    # shift for overflow-safe unnormalized softmax: |scores| <= sum|W_alpha|
    L = float(min(np.abs(W_alpha).sum() + abs(float(b_alpha[0])), 60.0))

    key = (has_bz, has_bz0, -L)
    if key not in _CACHE:
        _CACHE[key] = _build(has_bz, has_bz0, -L)
    nc = _CACHE[key]

    wa = W_alpha[:, 0]
    shared = {
        "wsl": _pack_w(W_sl), "whl": _pack_w(W_hl), "wse": _pack_w(W_se),
        "whe": _pack_w(W_he), "wctx": _pack_w(W_ctx2att), "watt": _pack_w(W_att2h),
        "wa4": np.ascontiguousarray(
            np.broadcast_to(wa.reshape(1, 1, D), (128, 4, D))).astype(BF16),
        "bsl": np.ascontiguousarray(b_sl.reshape(4, 128).T).astype(np.float32),
        "bhl": np.ascontiguousarray(b_hl.reshape(4, 128).T).astype(np.float32),
        "brep": np.ascontiguousarray(
            np.broadcast_to(b_att2h.reshape(1, D), (128, D))).astype(np.float32),
    }
    if has_bz:
        shared["bz"] = bias_z.reshape(1, D).astype(BF16)
    if has_bz0:
        shared["bz0"] = bias_z0.reshape(1, D).astype(BF16)

    in_maps = []
    for c in range(NCORES):
        sl = slice(c * BL, (c + 1) * BL)
        m = dict(shared)
        m["h"] = np.ascontiguousarray(h[sl])
        m["sent"] = np.ascontiguousarray(sentinel[sl])
        m["att"] = np.ascontiguousarray(att_feats[sl])
        in_maps.append(m)
    return nc, in_maps


def kernel(**inputs):
    from concourse.bass_utils import run_bass_kernel_spmd
    nc, in_maps = _prepare(**inputs)
    res = run_bass_kernel_spmd(nc, in_maps, list(range(NCORES)), trace=False)
    out = np.concatenate([res.results[i]["out"] for i in range(NCORES)], axis=0)
    return out.astype(np.float32)


# revision 13
# speedup vs baseline: 1.2095x; 1.2095x over previous
"""AdaAttention Trainium2 kernel: 8-way batch data parallel.

Full inputs in, full outputs out. Each of the 8 NeuronCores processes a
128-row batch shard. Weights (~1.3M params) are replicated, host-packed to
bf16 tiles.

Per-core dataflow:
  att_feats [128,196,512] f32 --(gpsimd casting DMA)--> bf16 natural tiles
    --(XBAR dma transpose)--> [d,b] stationary chunks
  z[b,s,h] = att@W_ctx (4 MM) + h_emb via one identity-MM (PSUM accumulation)
  hA = tanh(z)                                (ScalarE, batched x2 slices)
  scores = sum_h hA*W_alpha                   (DVE mult + grouped reduce)
  online softmax: e = exp(scores - L); num accumulated ON PE via
    diag(e) stationary matmuls into a dedicated PSUM bank; den = sum e
  out = tanh((num/den + h_lin) @ W_att2h + b) (PE + DVE + ScalarE)
"""
import numpy as np
import ml_dtypes

B = 1024
NCORES = 8
BL = B // NCORES          # 128 rows per core
S = 196                   # attention positions
D = 512                   # feature dim (RNN=ENC=HID=512)
SC = 14                   # att slices per DMA chunk
NCHUNK = S // SC          # 14
GRP = 2                   # slices per PSUM group
NGRP = SC // GRP          # 7 groups per chunk
CHUNKS = [SC] * NCHUNK

BF16 = ml_dtypes.bfloat16

_CACHE = {}


def _pack_w(w):
    # [512,512] (in,out) -> [128, 4, 512]: tile[p, dc, o] = w[dc*128+p, o]
    return np.ascontiguousarray(
        w.reshape(4, 128, D).transpose(1, 0, 2)).astype(BF16)


def _build(has_bz, has_bz0, neg_l):
    import concourse.bass as bass
    import concourse.tile as tile
    from concourse import bacc, mybir
    from concourse.masks import make_identity

    f32 = mybir.dt.float32
    bf16 = mybir.dt.bfloat16
    AF = mybir.ActivationFunctionType

    nc = bacc.Bacc("TRN2", target_bir_lowering=False, debug=False,
                   num_devices=NCORES)

    h_ap = nc.dram_tensor("h", [BL, D], f32, kind="ExternalInput").ap()
    sent_ap = nc.dram_tensor("sent", [BL, D], f32, kind="ExternalInput").ap()
    att_ap = nc.dram_tensor("att", [BL, S, D], f32, kind="ExternalInput").ap()
    w_aps = {}
    for name in ("wsl", "whl", "wse", "whe", "wctx", "watt", "wa4"):
        w_aps[name] = nc.dram_tensor(name, [128, 4, D], bf16,
                                     kind="ExternalInput").ap()
    bsl_ap = nc.dram_tensor("bsl", [128, 4], f32, kind="ExternalInput").ap()
    bhl_ap = nc.dram_tensor("bhl", [128, 4], f32, kind="ExternalInput").ap()
    brep_ap = nc.dram_tensor("brep", [128, D], f32, kind="ExternalInput").ap()
    if has_bz:
        bz_ap = nc.dram_tensor("bz", [1, D], bf16, kind="ExternalInput").ap()
    if has_bz0:
        bz0_ap = nc.dram_tensor("bz0", [1, D], bf16, kind="ExternalInput").ap()
    out_ap = nc.dram_tensor("out", [BL, D], f32, kind="ExternalOutput").ap()

    with tile.TileContext(nc) as tc:
        with tc.tile_pool(name="const", bufs=1) as constp, \
             tc.tile_pool(name="attp", bufs=3) as attp, \
             tc.tile_pool(name="attT", bufs=3) as attTp, \
             tc.tile_pool(name="hAp", bufs=3) as hAp, \
             tc.tile_pool(name="wzp", bufs=3) as wzp, \
             tc.tile_pool(name="scp", bufs=3) as scp, \
             tc.tile_pool(name="diagp", bufs=4) as diagp, \
             tc.tile_pool(name="small", bufs=1) as smallp, \
             tc.tile_pool(name="psum", bufs=3, space="PSUM") as psump, \
             tc.tile_pool(name="psumt", bufs=1, space="PSUM") as psumtp, \
             tc.tile_pool(name="nacc", bufs=1, space="PSUM") as naccp:

            # ---- h/sent casts first (tiny), then prefetch att chunk 0 ----
            h_bf = smallp.tile([128, D], bf16, tag="hbf")
            nc.gpsimd.dma_start(out=h_bf[:], in_=h_ap[:])
            sent_bf = smallp.tile([128, D], bf16, tag="sentbf")
            nc.gpsimd.dma_start(out=sent_bf[:], in_=sent_ap[:])
            att_bf0 = attp.tile([128, SC, D], bf16, tag="attbf")
            nc.gpsimd.dma_start(out=att_bf0[:, 0:CHUNKS[0], :],
                                in_=att_ap[:, 0:CHUNKS[0], :])

            # ---- constants / weights ----
            w = {}
            for name in ("wsl", "whl", "wctx", "whe", "wse", "wa4", "watt"):
                t = constp.tile([128, 4, D], bf16, tag=name)
                nc.sync.dma_start(out=t[:], in_=w_aps[name][:])
                w[name] = t
            bsl = constp.tile([128, 4], f32, tag="bsl")
            nc.scalar.dma_start(out=bsl[:], in_=bsl_ap[:])
            bhl = constp.tile([128, 4], f32, tag="bhl")
            nc.scalar.dma_start(out=bhl[:], in_=bhl_ap[:])
            brep = constp.tile([128, D], f32, tag="brep")
            nc.scalar.dma_start(out=brep[:], in_=brep_ap[:])
            bz = bz0 = None
            if has_bz:
                bz = constp.tile([1, D], bf16, tag="bz")
                nc.scalar.dma_start(out=bz[:], in_=bz_ap[:])
            if has_bz0:
                bz0 = constp.tile([1, D], bf16, tag="bz0")
                nc.scalar.dma_start(out=bz0[:], in_=bz0_ap[:])
            if has_bz or has_bz0:
                ones_row = constp.tile([1, 128], bf16, tag="ones")
                nc.vector.memset(ones_row[:], 1.0)
            ident = constp.tile([128, 128], bf16, tag="ident")
            make_identity(nc, ident[:])
            lshift = constp.tile([128, 1], f32, tag="lshift")
            nc.vector.memset(lshift[:], neg_l)
            e_all = constp.tile([128, 200], f32, tag="eall")

            # ---- h / sentinel branch ----
            hT = smallp.tile([128, 4, 128], bf16, tag="hT")
            nc.sync.dma_start_transpose(out=hT[:], in_=h_bf[:])
            sentT = smallp.tile([128, 4, 128], bf16, tag="sentT")
            nc.sync.dma_start_transpose(out=sentT[:], in_=sent_bf[:])

            def linear_T(xT, wt, bias_t, func, out_tag):
                """[e(P),b] = func(w[:,dc,:].T @ xT + b): transposed-layout
                linear layer. Returns bf16 [128, 4, 128] tile."""
                pt = psump.tile([128, GRP, D], f32, tag="z")
                ptf = pt.rearrange("p a b -> p (a b)")
                outT = smallp.tile([128, 4, 128], bf16, tag=out_tag)
                for ec in range(4):
                    reg = ptf[:, ec * 128:(ec + 1) * 128]
                    for dc in range(4):
                        nc.tensor.matmul(
                            reg, lhsT=wt[:, dc, ec * 128:(ec + 1) * 128],
                            rhs=xT[:, dc, :],
                            start=(dc == 0), stop=(dc == 3))
                    nc.scalar.activation(outT[:, ec, :], reg, func,
                                         bias=bias_t[:, ec:ec + 1])
                return outT

            sent_linT = linear_T(sentT, w["wsl"], bsl, AF.Relu, "slinT")
            h_linT = linear_T(hT, w["whl"], bhl, AF.Tanh, "hlinT")

            # natural-layout copies (b on partitions)
            ptn = psumtp.tile([128, GRP, D], bf16, tag="zt")
            ptn_flat = ptn.rearrange("p a b -> p (a b)")
            for ec in range(4):
                nc.tensor.transpose(ptn_flat[:, ec * 128:(ec + 1) * 128],
                                    sent_linT[:, ec, :], ident[:])
            for ec in range(4):
                nc.tensor.transpose(ptn_flat[:, D + ec * 128:D + (ec + 1) * 128],
                                    h_linT[:, ec, :], ident[:])
            sent_lin_nat = smallp.tile([128, D], bf16, tag="slnat")
            nc.scalar.copy(out=sent_lin_nat[:], in_=ptn_flat[:, 0:D])
            h_lin_nat = smallp.tile([128, D], f32, tag="hlnat")
            nc.scalar.copy(out=h_lin_nat[:], in_=ptn_flat[:, D:2 * D])

            # h_emb precomputed once in natural layout, injected per slice
            # with ONE identity matmul into the slice's PSUM.
            def make_hemb(extra_bias, tag):
                pt = psump.tile([128, GRP, D], f32, tag="z")
                reg = pt[:, 0, :]
                for ec in range(4):
                    nc.tensor.matmul(reg, lhsT=h_linT[:, ec, :],
                                     rhs=w["whe"][:, ec, :],
                                     start=(ec == 0),
                                     stop=(ec == 3 and extra_bias is None))
                if extra_bias is not None:
                    nc.tensor.matmul(reg, lhsT=ones_row[:],
                                     rhs=extra_bias[:], start=False, stop=True)
                out = smallp.tile([128, D], bf16, tag=tag)
                nc.scalar.copy(out=out[:], in_=reg)
                return out

            hemb_att = make_hemb(bz, "hembA")
            hemb_sent = make_hemb(bz0, "hembS")

            # num accumulator: one PSUM bank, PE-accumulated over 197 slices
            nacc = naccp.tile([128, D], f32, tag="nacc")

            def mac(e_col, src_nat, start, stop):
                dg = diagp.tile([128, 128], bf16, tag="diag")
                nc.gpsimd.affine_select(
                    out=dg[:], in_=e_col.to_broadcast((128, 128)),
                    compare_op=mybir.AluOpType.is_equal,
                    fill=0.0, base=0, pattern=[[-1, 128]],
                    channel_multiplier=1)
                nc.tensor.matmul(nacc[:], lhsT=dg[:], rhs=src_nat,
                                 start=start, stop=stop)

            # ---- sentinel slice (e-column 196) ----
            zt0 = psump.tile([128, GRP, D], f32, tag="z")
            reg0 = zt0[:, 0, :]
            for ec in range(4):
                nc.tensor.matmul(reg0, lhsT=sent_linT[:, ec, :],
                                 rhs=w["wse"][:, ec, :],
                                 start=(ec == 0), stop=False)
            nc.tensor.matmul(reg0, lhsT=ident[:], rhs=hemb_sent[:],
                             start=False, stop=True)
            hA0 = hAp.tile([128, GRP, D], bf16, tag="hA")
            nc.scalar.activation(hA0[:, 0, :], reg0, AF.Tanh)
            wz0 = wzp.tile([128, GRP, D], bf16, tag="wz")
            nc.vector.tensor_mul(wz0[:, 0, :], hA0[:, 0, :], w["wa4"][:, 0, :])
            sc0 = scp.tile([128, GRP], f32, tag="sc")
            nc.vector.reduce_sum(out=sc0[:, 0:1], in_=wz0[:, 0, :],
                                 axis=mybir.AxisListType.X)
            nc.scalar.activation(e_all[:, 196:197], sc0[:, 0:1], AF.Exp,
                                 bias=lshift[:])
            mac(e_all[:, 196:197], sent_lin_nat[:], start=True, stop=False)

            # ---- main loop over att slices (MAC pipelined 1 group behind) ----
            pending = []

            def flush_pending(last=False):
                limit = 0 if last else 3 * GRP
                while len(pending) > limit:
                    e_col, src = pending.pop(0)
                    mac(e_col, src, start=False,
                        stop=(last and not pending))

            NG = S // GRP                     # 98 groups total
            GPC = SC // GRP                   # groups per chunk
            PAIR = 2 * GRP                    # slices per DVE batch
            att_tiles = {}
            hA_pair = None
            for gi in range(NG):
                ck, g = divmod(gi, GPC)
                if g == 0:
                    if ck == 0:
                        att_bf = att_bf0
                    else:
                        att_bf = attp.tile([128, SC, D], bf16, tag="attbf")
                        nc.gpsimd.dma_start(
                            out=att_bf[:], in_=att_ap[:, ck * SC:(ck + 1) * SC, :])
                    att_tiles[ck] = att_bf
                    at_T = attTp.tile([128, SC * 4, 128], bf16, tag="attT")
                    nc.sync.dma_start_transpose(
                        out=at_T[:], in_=att_bf.rearrange("p a b -> p (a b)"))
                gl = g * GRP
                sg = gi * GRP
                zt = psump.tile([128, GRP, D], f32, tag="z")
                for j in range(GRP):
                    reg = zt[:, j, :]
                    for dc in range(4):
                        nc.tensor.matmul(
                            reg, lhsT=at_T[:, (gl + j) * 4 + dc, :],
                            rhs=w["wctx"][:, dc, :],
                            start=(dc == 0), stop=False)
                    nc.tensor.matmul(reg, lhsT=ident[:], rhs=hemb_att[:],
                                     start=False, stop=True)
                flush_pending()
                if gi % 2 == 0:
                    hA_pair = hAp.tile([128, PAIR, D], bf16, tag="hA")
                    nc.scalar.activation(hA_pair[:, 0:GRP, :], zt[:], AF.Tanh)
                else:
                    nc.scalar.activation(hA_pair[:, GRP:PAIR, :], zt[:], AF.Tanh)
                    wz = wzp.tile([128, PAIR, D], bf16, tag="wz")
                    nc.vector.tensor_mul(wz[:], hA_pair[:], w["wa4"][:, 0:PAIR, :])
                    sc4 = scp.tile([128, PAIR], f32, tag="sc")
                    nc.vector.reduce_sum(out=sc4[:], in_=wz[:],
                                         axis=mybir.AxisListType.X)
                    sp = sg - GRP
                    nc.scalar.activation(e_all[:, sp:sp + PAIR], sc4[:], AF.Exp,
                                         bias=lshift[:])
                    for j in range(PAIR):
                        sj = sp + j
                        pending.append((e_all[:, sj:sj + 1],
                                        att_tiles[sj // SC][:, sj % SC, :]))
            flush_pending(last=True)

            # ---- epilogue ----
            den = smallp.tile([128, 1], f32, tag="den")
            nc.vector.reduce_sum(out=den[:], in_=e_all[:, 0:197],
                                 axis=mybir.AxisListType.X)
            rec = smallp.tile([128, 1], f32, tag="rec")
            nc.vector.reciprocal(out=rec[:], in_=den[:])
            chat = smallp.tile([128, D], f32, tag="chat")
            nc.vector.tensor_scalar_mul(chat[:], nacc[:], rec[:])
            atten = smallp.tile([128, D], f32, tag="atten")
            nc.vector.tensor_add(atten[:], chat[:], h_lin_nat[:])
            atten_bf = smallp.tile([128, D], bf16, tag="attenbf")
            nc.vector.tensor_copy(out=atten_bf[:], in_=atten[:])
            ptf = psumtp.tile([128, GRP, D], bf16, tag="zt")
            ptf_flat = ptf.rearrange("p a b -> p (a b)")
            for dc in range(4):
                nc.tensor.transpose(ptf_flat[:, dc * 128:(dc + 1) * 128],
                                    atten_bf[:, dc * 128:(dc + 1) * 128],
                                    ident[:])
            attenT = smallp.tile([128, 4, 128], bf16, tag="attenT")
            nc.scalar.copy(out=attenT.rearrange("p a b -> p (a b)"),
                           in_=ptf_flat[:, 0:D])
            zf = psump.tile([128, GRP, D], f32, tag="z")
            regf = zf[:, 0, :]
            for dc in range(4):
                nc.tensor.matmul(regf, lhsT=attenT[:, dc, :],
                                 rhs=w["watt"][:, dc, :],
                                 start=(dc == 0), stop=(dc == 3))
            zfb = smallp.tile([128, D], f32, tag="zfb")
            nc.vector.tensor_add(zfb[:], regf, brep[:])
            out_sb = smallp.tile([128, D], f32, tag="outsb")
            nc.scalar.activation(out_sb[:], zfb[:], AF.Tanh)
            nc.scalar.dma_start(out=out_ap[:], in_=out_sb[:])

    nc.compile()
    return nc


def _prepare(h, sentinel, att_feats, W_ctx2att, b_ctx2att, W_sl, b_sl,
             W_se, b_se, W_hl, b_hl, W_he, b_he, W_alpha, b_alpha,
             W_att2h, b_att2h):
    h = np.asarray(h, dtype=np.float32)
    sentinel = np.asarray(sentinel, dtype=np.float32)
    att_feats = np.asarray(att_feats, dtype=np.float32)
    to_np = lambda a: np.asarray(a, dtype=np.float32)
    W_ctx2att, b_ctx2att = to_np(W_ctx2att), to_np(b_ctx2att)
    W_sl, b_sl = to_np(W_sl), to_np(b_sl)
    W_se, b_se = to_np(W_se), to_np(b_se)
    W_hl, b_hl = to_np(W_hl), to_np(b_hl)
    W_he, b_he = to_np(W_he), to_np(b_he)
    W_alpha, b_alpha = to_np(W_alpha), to_np(b_alpha)
    W_att2h, b_att2h = to_np(W_att2h), to_np(b_att2h)

    bias_z = b_ctx2att + b_he          # added to every att slice's z
    bias_z0 = b_se + b_he              # added to the sentinel slice's z
    has_bz = bool(np.any(bias_z))
    has_bz0 = bool(np.any(bias_z0))
    # shift for overflow-safe unnormalized softmax: |scores| <= sum|W_alpha|
    L = float(min(np.abs(W_alpha).sum() + abs(float(b_alpha[0])), 60.0))

    key = (has_bz, has_bz0, -L)
    if key not in _CACHE:
        _CACHE[key] = _build(has_bz, has_bz0, -L)
    nc = _CACHE[key]

    wa = W_alpha[:, 0]
    shared = {
        "wsl": _pack_w(W_sl), "whl": _pack_w(W_hl), "wse": _pack_w(W_se),
        "whe": _pack_w(W_he), "wctx": _pack_w(W_ctx2att), "watt": _pack_w(W_att2h),
        "wa4": np.ascontiguousarray(
            np.broadcast_to(wa.reshape(1, 1, D), (128, 4, D))).astype(BF16),
        "bsl": np.ascontiguousarray(b_sl.reshape(4, 128).T).astype(np.float32),
        "bhl": np.ascontiguousarray(b_hl.reshape(4, 128).T).astype(np.float32),
        "brep": np.ascontiguousarray(
            np.broadcast_to(b_att2h.reshape(1, D), (128, D))).astype(np.float32),
    }
    if has_bz:
        shared["bz"] = bias_z.reshape(1, D).astype(BF16)
    if has_bz0:
        shared["bz0"] = bias_z0.reshape(1, D).astype(BF16)

    in_maps = []
    for c in range(NCORES):
        sl = slice(c * BL, (c + 1) * BL)
        m = dict(shared)
        m["h"] = np.ascontiguousarray(h[sl])
        m["sent"] = np.ascontiguousarray(sentinel[sl])
        m["att"] = np.ascontiguousarray(att_feats[sl])
        in_maps.append(m)
    return nc, in_maps


def kernel(**inputs):
    from concourse.bass_utils import run_bass_kernel_spmd
    nc, in_maps = _prepare(**inputs)
    res = run_bass_kernel_spmd(nc, in_maps, list(range(NCORES)), trace=False)
    out = np.concatenate([res.results[i]["out"] for i in range(NCORES)], axis=0)
    return out.astype(np.float32)
